# revision 1
# baseline (speedup 1.0000x reference)
"""Multi-head causal attention (B=2, S=2048, D=1024, H=16) on 8 TRN2 NeuronCores.

Sharding: core c in 0..7 handles batch b = c // 4 and local head group
g = c % 4 (global heads 4g .. 4g+3).  Tensor-parallel over heads: each core
computes its heads' Q/K/V projections, causal attention, and a partial
output projection (W_out rows for its heads).  Host sums the 4 partials per
batch and adds b_out.

Per-core device kernel (all matmuls fp32r = f32 with 11-bit mantissa):
  phase 1: qkT = W_qkv_shard.T @ x.T  (hd-on-partition layout)
           V   = x @ Wv_shard          (seq-on-partition layout, +ones col)
  phase 2: per (q-chunk, head-pair, ki-group):
           scoresT[k,q] = kT.T-tile @ qT  -> exp (ACT, scale=1/8)
           -> causal 0/1 mask (diag chunks) -> AV + denominator matmuls
           softmax normalization via reciprocal + ones-matmul broadcast
  phase 3: outT_partial = W_out_shard.T @ valuesT  -> DRAM
"""

from contextlib import ExitStack

import numpy as np

import concourse.bass as bass
import concourse.mybir as mybir
import concourse.tile as tile
from concourse import bass_utils

F32 = mybir.dt.float32
F32R = mybir.dt.float32r
EXP = mybir.ActivationFunctionType.Exp

B, S, D, H = 2, 2048, 1024, 16
HD = D // H          # 64
HL = 4               # heads per core
N_CORES = 8
SC = S // 512        # 4 q-chunks of 512
KT = S // 128        # 16 k-tiles of 128

_CACHE = {}


def _round_f32r(x: np.ndarray) -> np.ndarray:
    """Round f32 to fp32r (11-bit mantissa, RNE) on host."""
    u = np.ascontiguousarray(x, dtype=np.float32).view(np.uint32)
    frac = u & np.uint32(0x00000FFF)
    base = u & np.uint32(0xFFFFF000)
    bit = np.uint32(0x00000800)
    lsb = np.uint32(0x00001000)
    roundup = (frac > bit) | ((frac == bit) & ((u & lsb) != 0))
    return np.where(roundup, base + lsb, base).view(np.float32)


_NO_HOIST = {
    "AllEngineBarrier",
    "EventSemaphore",
    "UnconditionalBranch",
    "CompareAndBranch",
    "BranchHint",
    "IndirectBranch",
    "Halt",
    "Call",
    "OverlayCall",
    "NoOp",
}


def _fix_sync_waits(nc):
    """walrus codegen holds only one sync-wait per engine instruction; hoist
    excess waits onto same-engine NoOps inserted right before."""
    for fn in nc.m.functions:
        for blk in fn.blocks:
            insts = blk.instructions
            out = []
            changed = False
            for inst in insts:
                si = inst.sync_info
                if si is not None and inst.opcode not in _NO_HOIST:
                    waits = list(si.on_wait)
                    if len(waits) > 1:
                        for j, w in enumerate(waits[:-1]):
                            nop = mybir.InstNoOp(name=f"{inst.name}-wfix{j}")
                            nop.engine = inst.engine
                            nop.sync_info = mybir.SyncInfo(on_wait=[w], on_update=[])
                            out.append(nop)
                        inst.sync_info = mybir.SyncInfo(
                            on_wait=[waits[-1]], on_update=list(si.on_update)
                        )
                        changed = True
                out.append(inst)
            if changed:
                blk.instructions = out


def _build(reps=1, fix_waits=True, n_chunks=SC, trim=False, masks=True):
    nc = bass.Bass("TRN2", target_bir_lowering=False, debug=False,
                   num_devices=N_CORES)

    xT = nc.dram_tensor("xT", [128, 8, S], F32R, kind="ExternalInput").ap()
    w = nc.dram_tensor("w", [128, 8, 768], F32R, kind="ExternalInput").ap()
    wout = nc.dram_tensor("wout", [128, 2, D], F32R, kind="ExternalInput").ap()
    bq = nc.dram_tensor("bq", [128, 4], F32, kind="ExternalInput").ap()
    bv = nc.dram_tensor("bv", [128, 256], F32, kind="ExternalInput").ap()
    vaug = nc.dram_tensor("vaug", [128, KT, HL, 1], F32R, kind="ExternalInput").ap()
    sel = nc.dram_tensor("sel", [128, 256], F32R, kind="ExternalInput").ap()
    cmask = nc.dram_tensor("cmask", [128, 4, 512], F32R, kind="ExternalInput").ap()
    outT = nc.dram_tensor("outT", [128, 8, S], F32, kind="ExternalOutput").ap()

    with tile.TileContext(nc) as tc, ExitStack() as ctx:
        persist = ctx.enter_context(tc.tile_pool(name="persist", bufs=1))
        xpool = ctx.enter_context(tc.tile_pool(name="xp", bufs=3))
        epool = ctx.enter_context(tc.tile_pool(name="ep", bufs=3))
        rpool = ctx.enter_context(tc.tile_pool(name="rp", bufs=2))
        ps_s = ctx.enter_context(tc.tile_pool(name="ps_s", bufs=2, space="PSUM"))
        ps_av = ctx.enter_context(tc.tile_pool(name="ps_av", bufs=2, space="PSUM"))
        # rb + outproj tiles share one double-buffered 2-bank slot set
        ps_rb = ctx.enter_context(tc.tile_pool(name="ps_rb", bufs=2, space="PSUM"))

        w_sb = persist.tile([128, 8, 768], F32R, tag="w")
        wout_sb = persist.tile([128, 2, D], F32R, tag="wout")
        bq_sb = persist.tile([128, 4], F32, tag="bq")
        bv_sb = persist.tile([128, 256], F32, tag="bv")
        sel_sb = persist.tile([128, 256], F32R, tag="sel")
        cmask_sb = persist.tile([128, 4, 512], F32R, tag="cmask")
        qT = persist.tile([128, 2, S], F32R, tag="qT")
        kT = persist.tile([128, 2, S], F32R, tag="kT")
        vn = persist.tile([128, KT, HL, 65], F32R, tag="vn")
        vraw = persist.tile([128, 2, S], F32, tag="vraw")
        vnorm = persist.tile([128, 2, S], F32R, tag="vnorm")

        # first x chunk + the W columns it needs come first, split into
        # k-halves across both DGE rings so the first matmuls start early
        xc0 = xpool.tile([128, 8, 512], F32R, tag="xc", name="xc0")
        nc.sync.dma_start(xc0[:, 0:4, :], xT[:, 0:4, 0:512])
        nc.scalar.dma_start(w_sb[:, 0:4, 0:512], w[:, 0:4, 0:512])
        nc.sync.dma_start(xc0[:, 4:8, :], xT[:, 4:8, 0:512])
        nc.scalar.dma_start(w_sb[:, 4:8, 0:512], w[:, 4:8, 0:512])
        nc.scalar.dma_start(bq_sb[:], bq)
        nc.sync.dma_start(w_sb[:, :, 512:768], w[:, :, 512:768])
        nc.scalar.dma_start(bv_sb[:], bv)
        nc.scalar.dma_start(cmask_sb[:], cmask)
        nc.sync.dma_start(wout_sb[:], wout)
        nc.scalar.dma_start(sel_sb[:], sel)
        # ones column 64 of the augmented V (softmax denominators)
        nc.sync.dma_start(vn[:, :, :, 64:65], vaug)

        # ---- phase 1 (per chunk): QKV projections ----
        def qkv_chunk(nq):
            qs = slice(nq * 512, (nq + 1) * 512)
            if nq == 0:
                xc = xc0
            else:
                xc = xpool.tile([128, 8, 512], F32R, tag="xc", name=f"xc{nq}")
                nc.sync.dma_start(xc[:], xT[:, :, qs])
            # qT / kT : psum[cols 128m.., seq 512] = W-tile.T @ xT
            for mh in range(2):
                sp = ps_s.tile([128, 1024], F32, tag="s", name=f"spq{nq}_{mh}")
                for m2 in range(2):
                    m = 2 * mh + m2
                    pm = sp[:, m2 * 512:(m2 + 1) * 512]
                    for k in range(8):
                        nc.tensor.matmul(pm, w_sb[:, k, m * 128:(m + 1) * 128],
                                         xc[:, k, :], start=(k == 0), stop=(k == 7))
                    dest = qT[:, m, qs] if m < 2 else kT[:, m - 2, qs]
                    nc.vector.tensor_scalar_add(dest, pm, bq_sb[:, m:m + 1])
            # V (natural layout): psum[seq 128, vcols 256] = x-tile.T-free @ Wv
            for j in range(4):
                st = 4 * nq + j
                pv = ps_av.tile([128, 512], F32, tag="av", name=f"pv{st}")
                for k in range(8):
                    nc.tensor.matmul(pv[:, 0:256],
                                     xc[:, k, j * 128:(j + 1) * 128],
                                     w_sb[:, k, 512:768],
                                     start=(k == 0), stop=(k == 7))
                nc.vector.tensor_add(
                    vn[:, st, :, 0:64],
                    pv[:, 0:256].rearrange("p (h d) -> p h d", h=4),
                    bv_sb[:].rearrange("p (h d) -> p h d", h=4))

        # ---- phase 2 (per chunk): causal attention, head pairs packed ----
        def attn_qc(qc):
            qs = slice(qc * 512, (qc + 1) * 512)
            for hp in range(2):          # head pair: heads 2hp (vp=0), 2hp+1 (vp=64)
                po = [ps_av.tile([128, 512], F32, tag="av",
                                 name=f"po{qc}{hp}{i}") for i in range(2)]
                n_ki = 4 * qc + 4
                es_hold = [None] * n_ki  # software pipeline: AV lags scores by 1

                # exact-ish causal at 256-q granularity (so both heads' score
                # blocks stay inside single PSUM banks): diagonal chunk
                # ki=4qc+j only touches q in [256*(j//2), 512) of the chunk;
                # the invalid prefix is masked with [zeros|triangle].
                def q_off(ki, qc=qc):
                    if not trim:
                        return 0
                    j = ki - 4 * qc
                    return 0 if j < 0 else 256 * (j // 2)

                def do_av(ki, qc=qc, hp=hp, po=po, n_ki=n_ki, es_hold=es_hold):
                    e = es_hold[ki]
                    o = q_off(ki)
                    wdt = 512 - o
                    for i in range(2):
                        h = 2 * hp + i
                        # [65,w] = V_aug.T @ E: rows 0..63 values, row 64
                        # the softmax denominator (ones column of V_aug)
                        nc.tensor.matmul(
                            po[i][0:65, o:512], vn[:, ki, h, 0:65],
                            e[:, i * wdt:(i + 1) * wdt],
                            start=(ki == 0), stop=(ki == n_ki - 1),
                            skip_group_check=True)

                for ki in range(n_ki):
                    ks = slice(ki * 128, (ki + 1) * 128)
                    o = q_off(ki)
                    wdt = 512 - o
                    qsub = slice(qc * 512 + o, (qc + 1) * 512)
                    sp = ps_s.tile([128, 1024], F32, tag="s",
                                   name=f"sp{qc}{hp}{ki}")
                    for i in range(2):   # head within pair (row-packed)
                        vp = i * 64
                        nc.tensor.matmul(
                            sp[:, i * wdt:(i + 1) * wdt],
                            kT[vp:vp + 64, hp, ks], qT[vp:vp + 64, hp, qsub],
                            start=True, stop=True, tile_position=(vp, 0))
                    e = epool.tile([128, 1024], F32R, tag="e",
                                   name=f"e{qc}{hp}{ki}")
                    nc.scalar.activation(e[:, 0:2 * wdt], sp[:, 0:2 * wdt],
                                         EXP, scale=0.125)
                    if masks and ki >= 4 * qc:   # diagonal: mask invalid prefix
                        # 0/1 mask over the first 128*(j+1) cols (rest is all
                        # valid); c_j[kk, qq] = 1 iff 128j + kk <= qq
                        j = ki - 4 * qc - (o // 128)
                        mw = 128 * (j + 1)
                        for i in range(2):
                            es = e[:, i * wdt:i * wdt + mw]
                            nc.vector.tensor_mul(es, es, cmask_sb[:, j, 0:mw])
                    es_hold[ki] = e
                    if ki >= 1:
                        do_av(ki - 1)
                do_av(n_ki - 1)
                # normalization: reciprocal of denominators (row 64 of each
                # po) + two accumulating K=1 fp32r sel-matmuls broadcasting
                # them onto their head's partitions
                recr = rpool.tile([128, 2, 512], F32R, tag="recr", name=f"rr{qc}{hp}")
                stage = rpool.tile([128, 512], F32, tag="stage", name=f"st{qc}{hp}")
                rb = ps_rb.tile([128, 512], F32, tag="rbo", name=f"rb{qc}{hp}")
                for i in range(2):
                    with nc.allow_low_precision(reason="fp32r softmax recip"):
                        nc.vector.reciprocal(recr[64:65, i, :], po[i][64:65, :])
                    if i == 0:
                        nc.scalar.copy(vraw[0:64, hp, qs], po[i][0:64, :])
                    else:
                        # odd head: values must land on partitions 64..127
                        nc.scalar.copy(stage[0:64, :], po[i][0:64, :])
                        nc.sync.dma_start(vraw[64:128, hp, qs], stage[0:64, :])
                for i in range(2):
                    nc.tensor.matmul(rb[:], sel_sb[64:65, i * 128:(i + 1) * 128],
                                     recr[64:65, i, :], start=(i == 0),
                                     stop=(i == 1), tile_position=(64, 0))
                nc.vector.tensor_mul(vnorm[:, hp, qs], vraw[:, hp, qs], rb[:])

        # ---- phase 3 (per chunk): partial output projection ----
        def outproj_qc(qc):
            qs = slice(qc * 512, (qc + 1) * 512)
            for m in range(8):
                pu = ps_rb.tile([128, 512], F32, tag="rbo", name=f"pu{qc}{m}")
                for t in range(2):
                    nc.tensor.matmul(pu[:], wout_sb[:, t, m * 128:(m + 1) * 128],
                                     vnorm[:, t, qs], start=(t == 0), stop=(t == 1))
                ou = xpool.tile([128, 512], F32, tag="ou", name=f"ou{qc}{m}")
                nc.vector.tensor_copy(ou[:], pu[:])
                nc.sync.dma_start(outT[:, m, qs], ou[:])

        # interleave: attention for q-chunk c needs exactly QKV chunks 0..c,
        # so qkv(c) -> attn(c) -> outproj(c) keeps every engine streaming
        for _rep in range(reps):
            for c in range(n_chunks):
                qkv_chunk(c)
                attn_qc(c)
                outproj_qc(c)

    if fix_waits:
        _fix_sync_waits(nc)
    return nc


def _get_nc():
    if "nc" not in _CACHE:
        _CACHE["nc"] = _build()
    return _CACHE["nc"]


def _make_masks() -> np.ndarray:
    """cmask[128, 4, 512]: c_j[kk, qq] = 1 iff 128j + kk <= qq (only the
    first 128*(j+1) cols are ever read; beyond that c_j is all-ones)."""
    kk = np.arange(128)[:, None]
    qq = np.arange(512)[None, :]
    return np.stack(
        [(128 * j + kk <= qq).astype(np.float32) for j in range(4)], axis=1)


def _make_vaug() -> np.ndarray:
    """vaug[128, KT, HL, 1]: the all-ones column 64 of the augmented V."""
    return np.ones((128, KT, HL, 1), np.float32)


def _make_sel() -> np.ndarray:
    """sel[128, 256]: broadcast matmul lhsTs at row 64 — col block 0 selects
    partitions 0..63 (even head), block 1 selects 64..127 (odd head)."""
    s = np.zeros((128, 256), np.float32)
    s[64, 0:64] = 1.0
    s[64, 192:256] = 1.0
    return s


def kernel(x, W_qkv, b_qkv, W_out, b_out):
    x = np.asarray(x, np.float32)
    W_qkv = np.asarray(W_qkv, np.float32)
    b_qkv = np.asarray(b_qkv, np.float32)
    W_out = np.asarray(W_out, np.float32)
    b_out = np.asarray(b_out, np.float32)

    nc = _get_nc()
    cmask = _make_masks()
    vaug = _make_vaug()
    selm = _make_sel()

    in_maps = []
    for c in range(N_CORES):
        b, g = divmod(c, 4)
        heads = [4 * g + i for i in range(HL)]
        # reorder W_qkv columns: [Q(h0..h3) | K(h0..h3) | V(h0..h3)]
        qcols = np.concatenate([W_qkv[:, h * 192:h * 192 + 64] for h in heads], 1)
        kcols = np.concatenate([W_qkv[:, h * 192 + 64:h * 192 + 128] for h in heads], 1)
        vcols = np.concatenate([W_qkv[:, h * 192 + 128:h * 192 + 192] for h in heads], 1)
        wsh = np.concatenate([qcols, kcols, vcols], 1)          # [1024, 768]
        bqv = np.concatenate([b_qkv[h * 192:h * 192 + 64] for h in heads])
        bkv = np.concatenate([b_qkv[h * 192 + 64:h * 192 + 128] for h in heads])
        bvv = np.concatenate([b_qkv[h * 192 + 128:h * 192 + 192] for h in heads])
        wo = W_out[g * 256:(g + 1) * 256, :]                    # [256, 1024]

        xT = x[b].T.reshape(8, 128, S).transpose(1, 0, 2)       # [128, 8, S]
        wsh3 = wsh.reshape(8, 128, 768).transpose(1, 0, 2)      # [128, 8, 768]
        wo3 = wo.reshape(2, 128, D).transpose(1, 0, 2)          # [128, 2, D]
        bq2 = np.concatenate([bqv, bkv]).reshape(4, 128).T      # [128, 4]
        bv2 = np.broadcast_to(bvv, (128, 256))                  # [128, 256]

        in_maps.append({
            "xT": _round_f32r(xT),
            "w": _round_f32r(wsh3),
            "wout": _round_f32r(wo3),
            "bq": np.ascontiguousarray(bq2),
            "bv": np.ascontiguousarray(bv2),
            "vaug": vaug,
            "sel": selm,
            "cmask": np.ascontiguousarray(cmask),
        })

    _CACHE["in_maps"] = in_maps
    res = bass_utils.run_bass_kernel_spmd(nc, in_maps, core_ids=list(range(N_CORES)))

    out = np.zeros((B, S, D), np.float32)
    for c in range(N_CORES):
        b = c // 4
        oT = res.results[c]["outT"]                             # [128, 8, S]
        out[b] += oT.transpose(1, 0, 2).reshape(D, S).T
    out += b_out
    return out



# revision 27
# speedup vs baseline: 1.3197x; 1.3197x over previous
"""Multi-head causal attention (B=2, S=2048, D=1024, H=16) on 8 TRN2 NeuronCores.

Sharding: core c in 0..7 handles batch b = c // 4 and local head group
g = c % 4 (global heads 4g .. 4g+3).  Tensor-parallel over heads: each core
computes its heads' Q/K/V projections, causal attention, and a partial
output projection (W_out rows for its heads).  Host sums the 4 partials per
batch and adds b_out.

v2: bf16 storage everywhere (f32 PSUM accumulation), 128-granularity causal
trim, k-sliced startup DMA with k-outer chunk-0 projection, and phase
interleaving: qkv(c+1) / outproj(c-1) matmul blocks are emitted as filler
between attention ki-steps of chunk c so the tensor engine never waits on
the activation-engine exp cadence.
"""

from contextlib import ExitStack

import numpy as np
import ml_dtypes

import concourse.bass as bass
import concourse.mybir as mybir
import concourse.tile as tile
from concourse import bass_utils

F32 = mybir.dt.float32
BF16 = mybir.dt.bfloat16
EXP = mybir.ActivationFunctionType.Exp

B, S, D, H = 2, 2048, 1024, 16
HD = D // H          # 64
HL = 4               # heads per core
N_CORES = 8
SC = S // 512        # 4 q-chunks of 512
KT = S // 128        # 16 k-tiles of 128

_CACHE = {}

_NO_HOIST = {
    "AllEngineBarrier",
    "EventSemaphore",
    "UnconditionalBranch",
    "CompareAndBranch",
    "BranchHint",
    "IndirectBranch",
    "Halt",
    "Call",
    "OverlayCall",
    "NoOp",
}


def _fix_sync_waits(nc):
    """walrus codegen holds only one sync-wait per engine instruction; hoist
    excess waits onto same-engine NoOps inserted right before."""
    for fn in nc.m.functions:
        for blk in fn.blocks:
            insts = blk.instructions
            out = []
            changed = False
            for inst in insts:
                si = inst.sync_info
                if si is not None and inst.opcode not in _NO_HOIST:
                    waits = list(si.on_wait)
                    if len(waits) > 1:
                        for j, w in enumerate(waits[:-1]):
                            nop = mybir.InstNoOp(name=f"{inst.name}-wfix{j}")
                            nop.engine = inst.engine
                            nop.sync_info = mybir.SyncInfo(on_wait=[w], on_update=[])
                            out.append(nop)
                        inst.sync_info = mybir.SyncInfo(
                            on_wait=[waits[-1]], on_update=list(si.on_update)
                        )
                        changed = True
                out.append(inst)
            if changed:
                blk.instructions = out


def _build(reps=1, fix_waits=True, n_chunks=SC, trim=True, masks=True):
    nc = bass.Bass("TRN2", target_bir_lowering=False, debug=False,
                   num_devices=N_CORES)

    xT = nc.dram_tensor("xT", [128, 8, S], BF16, kind="ExternalInput").ap()
    w = nc.dram_tensor("w", [128, 8, 768], BF16, kind="ExternalInput").ap()
    wout = nc.dram_tensor("wout", [128, 2, D], BF16, kind="ExternalInput").ap()
    bq = nc.dram_tensor("bq", [128, 4], F32, kind="ExternalInput").ap()
    bv = nc.dram_tensor("bv", [128, 256], F32, kind="ExternalInput").ap()
    cmask = nc.dram_tensor("cmask", [128, 128], BF16, kind="ExternalInput").ap()
    outT = nc.dram_tensor("outT", [128, 8, S], BF16, kind="ExternalOutput").ap()

    with tile.TileContext(nc) as tc, ExitStack() as ctx:
        persist = ctx.enter_context(tc.tile_pool(name="persist", bufs=1))
        xpool = ctx.enter_context(tc.tile_pool(name="xp", bufs=3))
        epool = ctx.enter_context(tc.tile_pool(name="ep", bufs=3))
        rpool = ctx.enter_context(tc.tile_pool(name="rp", bufs=2))
        opool = ctx.enter_context(tc.tile_pool(name="op", bufs=3))
        # PSUM: scores 2x[128,1024] (4 banks) + AV accum 2x[128,512] (2) +
        # shared qkv/outproj/rb pool 2x[128,512] (2) = 8 banks
        ps_sc = ctx.enter_context(tc.tile_pool(name="ps_sc", bufs=2, space="PSUM"))
        ps_av = ctx.enter_context(tc.tile_pool(name="ps_av", bufs=2, space="PSUM"))
        ps_mm = ctx.enter_context(tc.tile_pool(name="ps_mm", bufs=2, space="PSUM"))

        w_sb = persist.tile([128, 8, 768], BF16, tag="w")
        wout_sb = persist.tile([128, 2, D], BF16, tag="wout")
        bq_sb = persist.tile([128, 4], F32, tag="bq")
        bv_sb = persist.tile([128, 256], F32, tag="bv")
        sel_sb = persist.tile([128, 256], BF16, tag="sel")
        cmask_sb = persist.tile([128, 128], BF16, tag="cmask")
        qT = persist.tile([128, 2, S], BF16, tag="qT")
        kT = persist.tile([128, 2, S], BF16, tag="kT")
        vn = persist.tile([128, KT, HL, 65], BF16, tag="vn")
        vraw = persist.tile([128, 2, S], BF16, tag="vraw")
        vnorm = persist.tile([128, 2, S], BF16, tag="vnorm")

        # device-built constants: the softmax-denominator ones column (64) of
        # the augmented V, and the reciprocal-broadcast selector row: partition
        # 64, col block i selects the value partitions of head i of the pair.
        # (Engine APs must start at a mod-32 partition, so both heads'
        # denominators live on partition 64 at different free offsets.)
        for h in range(HL):
            nc.vector.memset(vn[:, :, h, 64:65], 1.0)
        nc.vector.memset(sel_sb[64:65, :], 0.0)
        nc.vector.memset(sel_sb[64:65, 0:64], 1.0)
        nc.vector.memset(sel_sb[64:65, 192:256], 1.0)

        # ---- startup DMA, k-sliced so the first matmuls start early ----
        xc0 = xpool.tile([128, 8, 512], BF16, tag="xc", name="xc0")
        nc.sync.dma_start(xc0[:, 0, :], xT[:, 0, 0:512])
        nc.scalar.dma_start(w_sb[:, 0, 0:512], w[:, 0, 0:512])
        nc.sync.dma_start(xc0[:, 1, :], xT[:, 1, 0:512])
        nc.scalar.dma_start(w_sb[:, 0, 512:768], w[:, 0, 512:768])
        nc.scalar.dma_start(w_sb[:, 1, :], w[:, 1, :])
        for k2 in range(1, 4):
            ks2 = slice(2 * k2, 2 * k2 + 2)
            nc.sync.dma_start(xc0[:, ks2, :], xT[:, ks2, 0:512])
            nc.scalar.dma_start(w_sb[:, ks2, :], w[:, ks2, :])
        nc.scalar.dma_start(bq_sb[:], bq)
        nc.scalar.dma_start(bv_sb[:], bv)
        nc.scalar.dma_start(cmask_sb[:], cmask)
        nc.scalar.dma_start(wout_sb[:], wout)

        # ---- chunk-0 qkv projection, k-outer (consumes slices as they land)
        # spA: [Q hp0 | K hp0], spB: [Q hp1 | K hp1]; pvA: [j0|j1], pvB: [j2|j3]
        # (hardware: at most ONE open matmul accumulation group per PSUM bank
        # — interleaved groups in a shared bank silently corrupt, so the four
        # qk accumulators get a bank each and V runs as sequential j-blocks)
        spA = ps_sc.tile([128, 1024], F32, tag="s", name="spA")
        spB = ps_sc.tile([128, 1024], F32, tag="s", name="spB")
        for k in range(8):
            st = k == 0
            sp_ = k == 7
            nc.tensor.matmul(spA[:, 0:512], w_sb[:, k, 0:128], xc0[:, k, :],
                             start=st, stop=sp_)
            nc.tensor.matmul(spA[:, 512:1024], w_sb[:, k, 256:384], xc0[:, k, :],
                             start=st, stop=sp_)
            nc.tensor.matmul(spB[:, 0:512], w_sb[:, k, 128:256], xc0[:, k, :],
                             start=st, stop=sp_)
            nc.tensor.matmul(spB[:, 512:1024], w_sb[:, k, 384:512], xc0[:, k, :],
                             start=st, stop=sp_)
        nc.vector.tensor_scalar_add(qT[:, 0, 0:512], spA[:, 0:512], bq_sb[:, 0:1])
        nc.vector.tensor_scalar_add(kT[:, 0, 0:512], spA[:, 512:1024], bq_sb[:, 2:3])
        nc.vector.tensor_scalar_add(qT[:, 1, 0:512], spB[:, 0:512], bq_sb[:, 1:2])
        nc.vector.tensor_scalar_add(kT[:, 1, 0:512], spB[:, 512:1024], bq_sb[:, 3:4])
        for st4 in range(4):
            pv = ps_av.tile([128, 512], F32, tag="av", name=f"pv0_{st4}")
            for k in range(8):
                nc.tensor.matmul(pv[:, 0:256],
                                 xc0[:, k, 128 * st4:128 * (st4 + 1)],
                                 w_sb[:, k, 512:768], start=(k == 0), stop=(k == 7))
            nc.vector.tensor_add(
                vn[:, st4, :, 0:64],
                pv[:, 0:256].rearrange("p (h d) -> p h d", h=4),
                bv_sb[:].rearrange("p (h d) -> p h d", h=4))

        xcs = {0: xc0}

        # ---- filler blocks: qkv projection of a later chunk / output
        # projection of an earlier chunk, emitted between attention steps ----
        def qk_halves(cn, m):
            # m: 0=Q hp0, 1=Q hp1, 2=K hp0, 3=K hp1 (matches w col + bias col)
            # split into two 4-k-step closures for finer filler granularity
            hold = {}

            def emit_a():
                hold["pm"] = ps_mm.tile([128, 512], F32, tag="mm",
                                        name=f"qk{cn}_{m}")
                for k in range(4):
                    nc.tensor.matmul(hold["pm"][:], w_sb[:, k, 128 * m:128 * (m + 1)],
                                     xcs[cn][:, k, :], start=(k == 0), stop=False)

            def emit_b():
                qs = slice(cn * 512, (cn + 1) * 512)
                pm = hold["pm"]
                for k in range(4, 8):
                    nc.tensor.matmul(pm[:], w_sb[:, k, 128 * m:128 * (m + 1)],
                                     xcs[cn][:, k, :], start=False, stop=(k == 7))
                dst = qT[:, m, qs] if m < 2 else kT[:, m - 2, qs]
                nc.vector.tensor_scalar_add(dst, pm[:], bq_sb[:, m:m + 1])
            return [emit_a, emit_b]

        def v_block(cn, j):
            def emit():
                pv = ps_mm.tile([128, 512], F32, tag="mm", name=f"v{cn}_{j}")
                for k in range(8):
                    nc.tensor.matmul(pv[:, 0:256],
                                     xcs[cn][:, k, 128 * j:128 * (j + 1)],
                                     w_sb[:, k, 512:768], start=(k == 0), stop=(k == 7))
                st4 = 4 * cn + j
                nc.vector.tensor_add(
                    vn[:, st4, :, 0:64],
                    pv[:, 0:256].rearrange("p (h d) -> p h d", h=4),
                    bv_sb[:].rearrange("p (h d) -> p h d", h=4))
            return emit

        ou_hold = {}

        def outproj_block(cn, m, copy_eng="dve"):
            # even m allocates a 2-block staging tile; odd m completes it and
            # issues one paired DMA (halves the per-transfer HWDGE overhead)
            def emit():
                qs = slice(cn * 512, (cn + 1) * 512)
                pu = ps_mm.tile([128, 512], F32, tag="mm", name=f"pu{cn}_{m}")
                for t in range(2):
                    nc.tensor.matmul(pu[:], wout_sb[:, t, 128 * m:128 * (m + 1)],
                                     vnorm[:, t, qs], start=(t == 0), stop=(t == 1))
                if m % 2 == 0:
                    ou_hold[cn] = opool.tile([128, 2, 512], BF16, tag="ou",
                                             name=f"ou{cn}_{m}")
                ou = ou_hold[cn]
                dst = ou[:, m % 2, :]
                if copy_eng == "dve":
                    nc.vector.tensor_copy(dst, pu[:])
                elif copy_eng == "act":
                    nc.scalar.copy(dst, pu[:])
                else:
                    nc.gpsimd.tensor_copy(dst, pu[:])
                if m % 2 == 1:
                    nc.sync.dma_start(outT[:, m - 1:m + 1, qs], ou[:])
            return emit

        # ---- attention for one q-chunk, with filler drained between steps
        def attn_qc(qc, fillers, reserve=()):
            n_ki = 4 * qc + 4
            nsteps = 2 * n_ki + 2
            state = {"step": 0, "drained": 0}

            def tick(n=None):
                state["step"] += 1
                if n is None:
                    target = len(fillers) * state["step"] // nsteps
                else:
                    target = state["drained"] + n
                while state["drained"] < min(target, len(fillers)):
                    fillers[state["drained"]]()
                    state["drained"] += 1

            qs = slice(qc * 512, (qc + 1) * 512)
            for hp in range(2):
                po = [ps_av.tile([128, 512], F32, tag="av",
                                 name=f"po{qc}{hp}{i}") for i in range(2)]
                es_hold = [None] * n_ki

                def do_av(ki, hp=hp, po=po, n_ki=n_ki, es_hold=es_hold):
                    e, o, wdt = es_hold[ki]
                    for i in range(2):
                        h = 2 * hp + i
                        # [65,w] = V_aug.T @ E: rows 0..63 values, row 64
                        # the softmax denominator (ones column of V_aug)
                        nc.tensor.matmul(
                            po[i][0:65, o:512], vn[:, ki, h, 0:65],
                            e[:, i * 512:i * 512 + wdt],
                            start=(ki == 0), stop=(ki == n_ki - 1),
                            skip_group_check=True)

                for ki in range(n_ki):
                    j = ki - 4 * qc
                    o = 128 * j if (trim and j >= 0) else 0
                    wdt = 512 - o
                    ks = slice(ki * 128, (ki + 1) * 128)
                    qsub = slice(qc * 512 + o, (qc + 1) * 512)
                    # head slabs live at bank-aligned offsets i*512 — the two
                    # tile_position score groups must not share a PSUM bank
                    sp = ps_sc.tile([128, 1024], F32, tag="s",
                                    name=f"sp{qc}{hp}{ki}")
                    for i in range(2):   # head within pair (row-packed)
                        vp = 64 * i
                        nc.tensor.matmul(
                            sp[:, i * 512:i * 512 + wdt],
                            kT[vp:vp + 64, hp, ks], qT[vp:vp + 64, hp, qsub],
                            start=True, stop=True, tile_position=(vp, 0))
                    e = epool.tile([128, 1024], BF16, tag="e",
                                   name=f"e{qc}{hp}{ki}")
                    if wdt == 512:
                        nc.scalar.activation(e[:], sp[:], EXP, scale=0.125)
                    else:
                        sp3 = sp[:].rearrange("p (t q) -> p t q", t=2)
                        e3 = e[:].rearrange("p (t q) -> p t q", t=2)
                        nc.scalar.activation(e3[:, :, 0:wdt], sp3[:, :, 0:wdt],
                                             EXP, scale=0.125)
                    if masks and j >= 0:
                        # diagonal tile: with o=128j the invalid region is
                        # always the leading 128-col triangle (kk > qq)
                        mw = min(128, wdt)
                        for i in range(2):
                            es = e[:, i * 512:i * 512 + mw]
                            nc.vector.tensor_mul(es, es, cmask_sb[:, 0:mw])
                    es_hold[ki] = (e, o, wdt)
                    if ki >= 1:
                        do_av(ki - 1)
                    tick()
                do_av(n_ki - 1)
                # normalization: per-head reciprocal of the denominator rows,
                # broadcast onto value partitions via a K=2 selector matmul.
                # Reciprocals first on DVE; the odd-head stage copy precedes
                # the even-head copy because its consumer chain (DMA -> vnorm)
                # is longer; PE runs filler/reserve while the drains flow.
                recr = rpool.tile([128, 2, 512], BF16, tag="recr", name=f"rr{qc}{hp}")
                stage = rpool.tile([128, 512], BF16, tag="stage", name=f"st{qc}{hp}")
                with nc.allow_low_precision(reason="bf16 softmax recip"):
                    nc.vector.reciprocal(recr[64:65, 0, :], po[0][64:65, :])
                    nc.vector.reciprocal(recr[64:65, 1, :], po[1][64:65, :])
                nc.scalar.copy(stage[0:64, :], po[1][0:64, :])
                # scalar ring: issued right behind its producer on the same
                # engine, so it never head-of-line-blocks the out-DMA ring
                nc.scalar.dma_start(vraw[64:128, hp, qs], stage[0:64, :])
                nc.scalar.copy(vraw[0:64, hp, qs], po[0][0:64, :])
                res = list(reserve) if (hp == 1 and reserve) else []
                if res:
                    res[0]()
                    res[1]()
                else:
                    tick(n=1)
                rb = ps_mm.tile([128, 512], F32, tag="mm", name=f"rb{qc}{hp}")
                for i in range(2):
                    nc.tensor.matmul(rb[:], sel_sb[64:65, 128 * i:128 * (i + 1)],
                                     recr[64:65, i, :], start=(i == 0),
                                     stop=(i == 1), tile_position=(64, 0))
                for r in res[2:]:
                    r()
                nc.vector.tensor_mul(vnorm[:, hp, qs], vraw[:, hp, qs], rb[:])
                tick()
            # any fillers not yet drained
            while state["drained"] < len(fillers):
                fillers[state["drained"]]()
                state["drained"] += 1

        # ---- main schedule: attn(c) with qkv(c+1) as filler; all deferrable
        # output projections (chunks 0..2) land in attn(3), whose exp cadence
        # otherwise starves the tensor engine; outproj(3) is the tail.  In
        # attn(3) the copies ride the idle Pool engine so the DVE recip ->
        # vnorm chain stays short; the last few blocks are reserved to keep
        # the PE warm through the final normalization chain.
        for c in range(n_chunks):
            fillers, reserve = [], []
            if c + 1 < n_chunks:
                xc = xpool.tile([128, 8, 512], BF16, tag="xc", name=f"xc{c+1}")
                qsn = slice((c + 1) * 512, (c + 2) * 512)
                nc.sync.dma_start(xc[:, 0:4, :], xT[:, 0:4, qsn])
                nc.sync.dma_start(xc[:, 4:8, :], xT[:, 4:8, qsn])
                xcs[c + 1] = xc
                for m in range(4):
                    fillers += qk_halves(c + 1, m)
                fillers += [v_block(c + 1, j) for j in range(4)]
            else:
                # GPSIMD cannot touch PSUM, so drain copies ride DVE (fillers)
                # and Act (reserve, where the exp pipeline has already drained)
                for cn in range(n_chunks - 2):
                    fillers += [outproj_block(cn, m, "dve") for m in range(8)]
                fillers += [outproj_block(n_chunks - 2, m, "dve") for m in range(4)]
                reserve = [outproj_block(n_chunks - 2, m, "act") for m in range(4, 8)]
            attn_qc(c, fillers, reserve)
        for m in range(8):
            outproj_block(n_chunks - 1, m, copy_eng=("dve" if m % 2 else "act"))()

    if fix_waits:
        _fix_sync_waits(nc)
    return nc


def _get_nc():
    if "nc" not in _CACHE:
        _CACHE["nc"] = _build()
    return _CACHE["nc"]


def _make_cmask() -> np.ndarray:
    """cmask[128, 128]: c[kk, qq] = 1 iff kk <= qq (relative causal triangle
    applied to the leading 128 cols of every diagonal score tile)."""
    kk = np.arange(128)[:, None]
    qq = np.arange(128)[None, :]
    return (kk <= qq).astype(np.float32)


def kernel(x, W_qkv, b_qkv, W_out, b_out):
    x = np.asarray(x, np.float32)
    W_qkv = np.asarray(W_qkv, np.float32)
    b_qkv = np.asarray(b_qkv, np.float32)
    W_out = np.asarray(W_out, np.float32)
    b_out = np.asarray(b_out, np.float32)

    nc = _get_nc()
    cmask = _make_cmask().astype(ml_dtypes.bfloat16)

    in_maps = []
    for c in range(N_CORES):
        b, g = divmod(c, 4)
        heads = [4 * g + i for i in range(HL)]
        # reorder W_qkv columns: [Q(h0..h3) | K(h0..h3) | V(h0..h3)]
        qcols = np.concatenate([W_qkv[:, h * 192:h * 192 + 64] for h in heads], 1)
        kcols = np.concatenate([W_qkv[:, h * 192 + 64:h * 192 + 128] for h in heads], 1)
        vcols = np.concatenate([W_qkv[:, h * 192 + 128:h * 192 + 192] for h in heads], 1)
        wsh = np.concatenate([qcols, kcols, vcols], 1)          # [1024, 768]
        bqv = np.concatenate([b_qkv[h * 192:h * 192 + 64] for h in heads])
        bkv = np.concatenate([b_qkv[h * 192 + 64:h * 192 + 128] for h in heads])
        bvv = np.concatenate([b_qkv[h * 192 + 128:h * 192 + 192] for h in heads])
        wo = W_out[g * 256:(g + 1) * 256, :]                    # [256, 1024]

        xT = x[b].T.reshape(8, 128, S).transpose(1, 0, 2)       # [128, 8, S]
        wsh3 = wsh.reshape(8, 128, 768).transpose(1, 0, 2)      # [128, 8, 768]
        wo3 = wo.reshape(2, 128, D).transpose(1, 0, 2)          # [128, 2, D]
        bq2 = np.concatenate([bqv, bkv]).reshape(4, 128).T      # [128, 4]
        bv2 = np.broadcast_to(bvv, (128, 256))                  # [128, 256]

        in_maps.append({
            "xT": np.ascontiguousarray(xT).astype(ml_dtypes.bfloat16),
            "w": np.ascontiguousarray(wsh3).astype(ml_dtypes.bfloat16),
            "wout": np.ascontiguousarray(wo3).astype(ml_dtypes.bfloat16),
            "bq": np.ascontiguousarray(bq2),
            "bv": np.ascontiguousarray(bv2),
            "cmask": np.ascontiguousarray(cmask),
        })

    _CACHE["in_maps"] = in_maps
    res = bass_utils.run_bass_kernel_spmd(nc, in_maps, core_ids=list(range(N_CORES)))

    out = np.zeros((B, S, D), np.float32)
    for c in range(N_CORES):
        b = c // 4
        oT = np.asarray(res.results[c]["outT"]).astype(np.float32)  # [128, 8, S]
        out[b] += oT.transpose(1, 0, 2).reshape(D, S).T
    out += b_out
    return out


# revision 48
# speedup vs baseline: 1.3312x; 1.0087x over previous
"""Multi-head causal attention (B=2, S=2048, D=1024, H=16) on 8 TRN2 NeuronCores.

Sharding: core c in 0..7 handles batch b = c // 4 and local head group
g = c % 4 (global heads 4g .. 4g+3).  Tensor-parallel over heads: each core
computes its heads' Q/K/V projections, causal attention, and a partial
output projection (W_out rows for its heads).  Host sums the 4 partials per
batch and adds b_out.

v2: bf16 storage everywhere (f32 PSUM accumulation), 128-granularity causal
trim, k-sliced startup DMA with k-outer chunk-0 projection, and phase
interleaving: qkv(c+1) / outproj(c-1) matmul blocks are emitted as filler
between attention ki-steps of chunk c so the tensor engine never waits on
the activation-engine exp cadence.
"""

from contextlib import ExitStack

import numpy as np
import ml_dtypes

import concourse.bass as bass
import concourse.mybir as mybir
import concourse.tile as tile
from concourse import bass_utils

F32 = mybir.dt.float32
BF16 = mybir.dt.bfloat16
EXP = mybir.ActivationFunctionType.Exp

B, S, D, H = 2, 2048, 1024, 16
HD = D // H          # 64
HL = 4               # heads per core
N_CORES = 8
SC = S // 512        # 4 q-chunks of 512
KT = S // 128        # 16 k-tiles of 128

_CACHE = {}

_NO_HOIST = {
    "AllEngineBarrier",
    "EventSemaphore",
    "UnconditionalBranch",
    "CompareAndBranch",
    "BranchHint",
    "IndirectBranch",
    "Halt",
    "Call",
    "OverlayCall",
    "NoOp",
}


def _fix_sync_waits(nc):
    """walrus codegen holds only one sync-wait per engine instruction; hoist
    excess waits onto same-engine NoOps inserted right before."""
    for fn in nc.m.functions:
        for blk in fn.blocks:
            insts = blk.instructions
            out = []
            changed = False
            for inst in insts:
                si = inst.sync_info
                if si is not None and inst.opcode not in _NO_HOIST:
                    waits = list(si.on_wait)
                    if len(waits) > 1:
                        for j, w in enumerate(waits[:-1]):
                            nop = mybir.InstNoOp(name=f"{inst.name}-wfix{j}")
                            nop.engine = inst.engine
                            nop.sync_info = mybir.SyncInfo(on_wait=[w], on_update=[])
                            out.append(nop)
                        inst.sync_info = mybir.SyncInfo(
                            on_wait=[waits[-1]], on_update=list(si.on_update)
                        )
                        changed = True
                out.append(inst)
            if changed:
                blk.instructions = out


def _build(reps=1, fix_waits=True, n_chunks=SC, trim=True, masks=True):
    nc = bass.Bass("TRN2", target_bir_lowering=False, debug=False,
                   num_devices=N_CORES)

    xT = nc.dram_tensor("xT", [128, 8, S], BF16, kind="ExternalInput").ap()
    w = nc.dram_tensor("w", [128, 8, 768], BF16, kind="ExternalInput").ap()
    wout = nc.dram_tensor("wout", [128, 2, D], BF16, kind="ExternalInput").ap()
    bq = nc.dram_tensor("bq", [128, 4], F32, kind="ExternalInput").ap()
    bv = nc.dram_tensor("bv", [128, 256], F32, kind="ExternalInput").ap()
    cmask = nc.dram_tensor("cmask", [128, 128], BF16, kind="ExternalInput").ap()
    outT = nc.dram_tensor("outT", [128, 8, S], BF16, kind="ExternalOutput").ap()

    with tile.TileContext(nc) as tc, ExitStack() as ctx:
        persist = ctx.enter_context(tc.tile_pool(name="persist", bufs=1))
        xpool = ctx.enter_context(tc.tile_pool(name="xp", bufs=3))
        epool = ctx.enter_context(tc.tile_pool(name="ep", bufs=3))
        rpool = ctx.enter_context(tc.tile_pool(name="rp", bufs=2))
        opool = ctx.enter_context(tc.tile_pool(name="op", bufs=3))
        # PSUM: scores 2x[128,1024] (4 banks) + AV accum 2x[128,512] (2) +
        # shared qkv/outproj/rb pool 2x[128,512] (2) = 8 banks
        ps_sc = ctx.enter_context(tc.tile_pool(name="ps_sc", bufs=2, space="PSUM"))
        ps_av = ctx.enter_context(tc.tile_pool(name="ps_av", bufs=2, space="PSUM"))
        ps_mm = ctx.enter_context(tc.tile_pool(name="ps_mm", bufs=2, space="PSUM"))

        w_sb = persist.tile([128, 8, 768], BF16, tag="w")
        wout_sb = persist.tile([128, 2, D], BF16, tag="wout")
        bq_sb = persist.tile([128, 4], F32, tag="bq")
        bv_sb = persist.tile([128, 256], F32, tag="bv")
        sel_sb = persist.tile([128, 256], BF16, tag="sel")
        cmask_sb = persist.tile([128, 128], BF16, tag="cmask")
        qT = persist.tile([128, 2, S], BF16, tag="qT")
        kT = persist.tile([128, 2, S], BF16, tag="kT")
        vn = persist.tile([128, KT, HL, 97], BF16, tag="vn")
        vraw = persist.tile([128, 2, S], BF16, tag="vraw")
        vnorm = persist.tile([128, 2, S], BF16, tag="vnorm")

        # device-built constants: the softmax-denominator ones column of the
        # augmented V — col 64 for even heads (den -> po row 64), col 96 for
        # odd heads (den -> po row 96, a valid mod-32 partition base) — and
        # the K=33 reciprocal-broadcast selector (rows 65..95 all zero).
        for h in range(HL):
            if h % 2 == 0:
                nc.vector.memset(vn[:, :, h, 64:65], 1.0)
            else:
                nc.vector.memset(vn[:, :, h, 64:96], 0.0)
                nc.vector.memset(vn[:, :, h, 96:97], 1.0)
        nc.vector.memset(sel_sb[64:96, 0:128], 0.0)
        nc.vector.memset(sel_sb[64:65, 0:64], 1.0)
        nc.vector.memset(sel_sb[96:97, 0:64], 0.0)
        nc.vector.memset(sel_sb[96:97, 64:128], 1.0)
        recrs = [persist.tile([128, 512], BF16, tag=f"recr{i}", name=f"recr{i}")
                 for i in range(2)]
        for r in recrs:
            nc.vector.memset(r[64:96, :], 0.0)

        # ---- startup DMA, k-sliced so the first matmuls start early ----
        xc0 = xpool.tile([128, 8, 512], BF16, tag="xc", name="xc0")
        nc.sync.dma_start(xc0[:, 0, :], xT[:, 0, 0:512])
        nc.scalar.dma_start(w_sb[:, 0, 0:512], w[:, 0, 0:512])
        nc.sync.dma_start(xc0[:, 1, :], xT[:, 1, 0:512])
        nc.scalar.dma_start(w_sb[:, 0, 512:768], w[:, 0, 512:768])
        nc.scalar.dma_start(w_sb[:, 1, :], w[:, 1, :])
        for k2 in range(1, 4):
            ks2 = slice(2 * k2, 2 * k2 + 2)
            nc.sync.dma_start(xc0[:, ks2, :], xT[:, ks2, 0:512])
            nc.scalar.dma_start(w_sb[:, ks2, :], w[:, ks2, :])
        nc.scalar.dma_start(bq_sb[:], bq)
        nc.scalar.dma_start(bv_sb[:], bv)
        nc.scalar.dma_start(cmask_sb[:], cmask)
        nc.scalar.dma_start(wout_sb[:], wout)

        # ---- chunk-0 qkv projection, k-outer (consumes slices as they land)
        # spA: [Q hp0 | K hp0], spB: [Q hp1 | K hp1]; pvA: [j0|j1], pvB: [j2|j3]
        # (hardware: at most ONE open matmul accumulation group per PSUM bank
        # — interleaved groups in a shared bank silently corrupt, so the four
        # qk accumulators get a bank each and V runs as sequential j-blocks)
        spA = ps_sc.tile([128, 1024], F32, tag="s", name="spA")
        spB = ps_sc.tile([128, 1024], F32, tag="s", name="spB")
        for k in range(8):
            st = k == 0
            sp_ = k == 7
            nc.tensor.matmul(spA[:, 0:512], w_sb[:, k, 0:128], xc0[:, k, :],
                             start=st, stop=sp_)
            nc.tensor.matmul(spA[:, 512:1024], w_sb[:, k, 256:384], xc0[:, k, :],
                             start=st, stop=sp_)
            nc.tensor.matmul(spB[:, 0:512], w_sb[:, k, 128:256], xc0[:, k, :],
                             start=st, stop=sp_)
            nc.tensor.matmul(spB[:, 512:1024], w_sb[:, k, 384:512], xc0[:, k, :],
                             start=st, stop=sp_)
        nc.vector.tensor_scalar_add(qT[:, 0, 0:512], spA[:, 0:512], bq_sb[:, 0:1])
        nc.vector.tensor_scalar_add(kT[:, 0, 0:512], spA[:, 512:1024], bq_sb[:, 2:3])
        nc.vector.tensor_scalar_add(qT[:, 1, 0:512], spB[:, 0:512], bq_sb[:, 1:2])
        nc.vector.tensor_scalar_add(kT[:, 1, 0:512], spB[:, 512:1024], bq_sb[:, 3:4])
        for st4 in range(4):
            pv = ps_av.tile([128, 512], F32, tag="av", name=f"pv0_{st4}")
            for k in range(8):
                nc.tensor.matmul(pv[:, 0:256],
                                 xc0[:, k, 128 * st4:128 * (st4 + 1)],
                                 w_sb[:, k, 512:768], start=(k == 0), stop=(k == 7))
            nc.vector.tensor_add(
                vn[:, st4, :, 0:64],
                pv[:, 0:256].rearrange("p (h d) -> p h d", h=4),
                bv_sb[:].rearrange("p (h d) -> p h d", h=4))

        xcs = {0: xc0}

        # ---- filler blocks: qkv projection of a later chunk / output
        # projection of an earlier chunk, emitted between attention steps ----
        def qk_halves(cn, m):
            # m: 0=Q hp0, 1=Q hp1, 2=K hp0, 3=K hp1 (matches w col + bias col)
            # split into two 4-k-step closures for finer filler granularity
            hold = {}

            def emit_a():
                hold["pm"] = ps_mm.tile([128, 512], F32, tag="mm",
                                        name=f"qk{cn}_{m}")
                for k in range(4):
                    nc.tensor.matmul(hold["pm"][:], w_sb[:, k, 128 * m:128 * (m + 1)],
                                     xcs[cn][:, k, :], start=(k == 0), stop=False)

            def emit_b():
                qs = slice(cn * 512, (cn + 1) * 512)
                pm = hold["pm"]
                for k in range(4, 8):
                    nc.tensor.matmul(pm[:], w_sb[:, k, 128 * m:128 * (m + 1)],
                                     xcs[cn][:, k, :], start=False, stop=(k == 7))
                dst = qT[:, m, qs] if m < 2 else kT[:, m - 2, qs]
                nc.vector.tensor_scalar_add(dst, pm[:], bq_sb[:, m:m + 1])
            return [emit_a, emit_b]

        def v_block(cn, j):
            def emit():
                pv = ps_mm.tile([128, 512], F32, tag="mm", name=f"v{cn}_{j}")
                for k in range(8):
                    nc.tensor.matmul(pv[:, 0:256],
                                     xcs[cn][:, k, 128 * j:128 * (j + 1)],
                                     w_sb[:, k, 512:768], start=(k == 0), stop=(k == 7))
                st4 = 4 * cn + j
                nc.vector.tensor_add(
                    vn[:, st4, :, 0:64],
                    pv[:, 0:256].rearrange("p (h d) -> p h d", h=4),
                    bv_sb[:].rearrange("p (h d) -> p h d", h=4))
            return emit

        ou_hold = {}

        def outproj_block(cn, m, copy_eng="dve"):
            # even m allocates a 2-block staging tile; odd m completes it and
            # issues one paired DMA (halves the per-transfer HWDGE overhead)
            def emit():
                qs = slice(cn * 512, (cn + 1) * 512)
                pu = ps_mm.tile([128, 512], F32, tag="mm", name=f"pu{cn}_{m}")
                for t in range(2):
                    nc.tensor.matmul(pu[:], wout_sb[:, t, 128 * m:128 * (m + 1)],
                                     vnorm[:, t, qs], start=(t == 0), stop=(t == 1))
                if m % 2 == 0:
                    ou_hold[cn] = opool.tile([128, 2, 512], BF16, tag="ou",
                                             name=f"ou{cn}_{m}")
                ou = ou_hold[cn]
                dst = ou[:, m % 2, :]
                if copy_eng == "dve":
                    nc.vector.tensor_copy(dst, pu[:])
                elif copy_eng == "act":
                    nc.scalar.copy(dst, pu[:])
                else:
                    # tail blocks: halve the drain latency by copying the two
                    # halves on DVE and Act in parallel
                    nc.vector.tensor_copy(dst[:, 0:256], pu[:, 0:256])
                    nc.scalar.copy(dst[:, 256:512], pu[:, 256:512])
                if m % 2 == 1:
                    nc.sync.dma_start(outT[:, m - 1:m + 1, qs], ou[:])
            return emit

        # pending normalization chain of the previous head pair — emitted
        # right AFTER the next head pair's first exp is queued, so the Act
        # engine starts the next exp before the drain copies, and the PE has
        # scores/filler work while the reciprocal chain flows (crosses chunk
        # boundaries too)
        pending = {"norm": None}

        # ---- attention for one q-chunk, with filler drained between steps
        def attn_qc(qc, fillers, reserve=()):
            n_ki = 4 * qc + 4
            nsteps = 2 * n_ki + 2
            state = {"step": 0, "drained": 0}

            def tick(n=None):
                state["step"] += 1
                if n is None:
                    target = len(fillers) * state["step"] // nsteps
                else:
                    target = state["drained"] + n
                while state["drained"] < min(target, len(fillers)):
                    fillers[state["drained"]]()
                    state["drained"] += 1

            qs = slice(qc * 512, (qc + 1) * 512)
            for hp in range(2):
                # po tiles are allocated lazily at the first AV so the pool
                # WAR lands after the previous pair's (deferred) drain copies
                po = [None, None]
                recr = recrs[(2 * qc + hp) % 2]
                es_hold = [None] * n_ki

                def do_av(ki, qc=qc, hp=hp, po=po, n_ki=n_ki, es_hold=es_hold):
                    if po[0] is None:
                        po[0] = ps_av.tile([128, 512], F32, tag="av",
                                           name=f"po{qc}{hp}0")
                        po[1] = ps_av.tile([128, 512], F32, tag="av",
                                           name=f"po{qc}{hp}1")
                    e, o, wdt = es_hold[ki]
                    for i in range(2):
                        h = 2 * hp + i
                        # V_aug.T @ E: rows 0..63 values, row 64 (even) or 96
                        # (odd) the softmax denominator (ones column of V_aug)
                        nc.tensor.matmul(
                            po[i][0:65 + 32 * i, o:512], vn[:, ki, h, 0:65 + 32 * i],
                            e[:, i * 512:i * 512 + wdt],
                            start=(ki == 0), stop=(ki == n_ki - 1),
                            skip_group_check=True)

                for ki in range(n_ki):
                    j = ki - 4 * qc
                    o = 128 * j if (trim and j >= 0) else 0
                    wdt = 512 - o
                    ks = slice(ki * 128, (ki + 1) * 128)
                    qsub = slice(qc * 512 + o, (qc + 1) * 512)
                    # head slabs live at bank-aligned offsets i*512 — the two
                    # tile_position score groups must not share a PSUM bank
                    sp = ps_sc.tile([128, 1024], F32, tag="s",
                                    name=f"sp{qc}{hp}{ki}")
                    for i in range(2):   # head within pair (row-packed)
                        vp = 64 * i
                        nc.tensor.matmul(
                            sp[:, i * 512:i * 512 + wdt],
                            kT[vp:vp + 64, hp, ks], qT[vp:vp + 64, hp, qsub],
                            start=True, stop=True, tile_position=(vp, 0))
                    e = epool.tile([128, 1024], BF16, tag="e",
                                   name=f"e{qc}{hp}{ki}")
                    if wdt == 512:
                        nc.scalar.activation(e[:], sp[:], EXP, scale=0.125)
                    else:
                        sp3 = sp[:].rearrange("p (t q) -> p t q", t=2)
                        e3 = e[:].rearrange("p (t q) -> p t q", t=2)
                        nc.scalar.activation(e3[:, :, 0:wdt], sp3[:, :, 0:wdt],
                                             EXP, scale=0.125)
                    if masks and j >= 0:
                        # diagonal tile: with o=128j the invalid region is
                        # always the leading 128-col triangle (kk > qq)
                        mw = min(128, wdt)
                        for i in range(2):
                            es = e[:, i * 512:i * 512 + mw]
                            nc.vector.tensor_mul(es, es, cmask_sb[:, 0:mw])
                    es_hold[ki] = (e, o, wdt)
                    if ki == 0 and pending["norm"] is not None:
                        pending["norm"]()
                        pending["norm"] = None
                    if ki >= 1:
                        do_av(ki - 1)
                    tick()
                do_av(n_ki - 1)
                tick()

                # normalization: per-head reciprocal of the denominator rows,
                # broadcast onto value partitions via one K=33 selector matmul.
                # Denominators: even head on po[0] row 64, odd head on po[1]
                # row 96 (the odd V_aug ones column sits at col 96), so both
                # reciprocals stay partition-aligned (rows 65..95 are zero).
                def norm(qc=qc, hp=hp, po=po, recr=recr, qs=qs,
                         res=tuple(reserve) if (hp == 1 and reserve) else ()):
                    stage = rpool.tile([128, 512], BF16, tag="stage",
                                       name=f"st{qc}{hp}")
                    nc.scalar.copy(stage[0:64, :], po[1][0:64, :])
                    # sync ring: out-DMAs queued behind this were emitted in
                    # the same window, so the short stage-copy wait cannot
                    # head-of-line-block them for long
                    nc.sync.dma_start(vraw[64:128, hp, qs], stage[0:64, :])
                    with nc.allow_low_precision(reason="bf16 softmax recip"):
                        nc.vector.reciprocal(recr[64:65, :], po[0][64:65, :])
                        nc.vector.reciprocal(recr[96:97, :], po[1][96:97, :])
                    nc.scalar.copy(vraw[0:64, hp, qs], po[0][0:64, :])
                    if res:
                        res[0]()
                        res[1]()
                    else:
                        tick(n=1)
                    rb = ps_mm.tile([128, 512], F32, tag="mm", name=f"rb{qc}{hp}")
                    nc.tensor.matmul(rb[:], sel_sb[64:97, 0:128], recr[64:97, :],
                                     start=True, stop=True, tile_position=(64, 0))
                    for r in res[2:]:
                        r()
                    nc.vector.tensor_mul(vnorm[:, hp, qs], vraw[:, hp, qs], rb[:])
                pending["norm"] = norm
            # any fillers not yet drained
            while state["drained"] < len(fillers):
                fillers[state["drained"]]()
                state["drained"] += 1

        # ---- main schedule: attn(c) with qkv(c+1) as filler; all deferrable
        # output projections (chunks 0..2) land in attn(3), whose exp cadence
        # otherwise starves the tensor engine; outproj(3) is the tail.  In
        # attn(3) the copies ride the idle Pool engine so the DVE recip ->
        # vnorm chain stays short; the last few blocks are reserved to keep
        # the PE warm through the final normalization chain.
        for c in range(n_chunks):
            fillers, reserve = [], []
            if c + 1 < n_chunks:
                xc = xpool.tile([128, 8, 512], BF16, tag="xc", name=f"xc{c+1}")
                qsn = slice((c + 1) * 512, (c + 2) * 512)
                nc.sync.dma_start(xc[:, 0:4, :], xT[:, 0:4, qsn])
                nc.sync.dma_start(xc[:, 4:8, :], xT[:, 4:8, qsn])
                xcs[c + 1] = xc
                for m in range(4):
                    fillers += qk_halves(c + 1, m)
                fillers += [v_block(c + 1, j) for j in range(4)]
            else:
                # GPSIMD cannot touch PSUM, so drain copies ride DVE (fillers)
                # and Act (reserve, where the exp pipeline has already drained)
                for cn in range(n_chunks - 2):
                    fillers += [outproj_block(cn, m, "dve") for m in range(8)]
                fillers += [outproj_block(n_chunks - 2, m, "dve") for m in range(4)]
                reserve = [outproj_block(n_chunks - 2, m, "act") for m in range(4, 8)]
            attn_qc(c, fillers, reserve)
        # the last head pair's normalization, then the tail output projection
        pending["norm"]()
        pending["norm"] = None
        for m in range(8):
            outproj_block(n_chunks - 1, m, copy_eng=("dve" if m % 2 else "act"))()

    if fix_waits:
        _fix_sync_waits(nc)
    return nc


def _get_nc():
    if "nc" not in _CACHE:
        _CACHE["nc"] = _build()
    return _CACHE["nc"]


def _make_cmask() -> np.ndarray:
    """cmask[128, 128]: c[kk, qq] = 1 iff kk <= qq (relative causal triangle
    applied to the leading 128 cols of every diagonal score tile)."""
    kk = np.arange(128)[:, None]
    qq = np.arange(128)[None, :]
    return (kk <= qq).astype(np.float32)


def kernel(x, W_qkv, b_qkv, W_out, b_out):
    x = np.asarray(x, np.float32)
    W_qkv = np.asarray(W_qkv, np.float32)
    b_qkv = np.asarray(b_qkv, np.float32)
    W_out = np.asarray(W_out, np.float32)
    b_out = np.asarray(b_out, np.float32)

    nc = _get_nc()
    cmask = _make_cmask().astype(ml_dtypes.bfloat16)

    in_maps = []
    for c in range(N_CORES):
        b, g = divmod(c, 4)
        heads = [4 * g + i for i in range(HL)]
        # reorder W_qkv columns: [Q(h0..h3) | K(h0..h3) | V(h0..h3)]
        qcols = np.concatenate([W_qkv[:, h * 192:h * 192 + 64] for h in heads], 1)
        kcols = np.concatenate([W_qkv[:, h * 192 + 64:h * 192 + 128] for h in heads], 1)
        vcols = np.concatenate([W_qkv[:, h * 192 + 128:h * 192 + 192] for h in heads], 1)
        wsh = np.concatenate([qcols, kcols, vcols], 1)          # [1024, 768]
        bqv = np.concatenate([b_qkv[h * 192:h * 192 + 64] for h in heads])
        bkv = np.concatenate([b_qkv[h * 192 + 64:h * 192 + 128] for h in heads])
        bvv = np.concatenate([b_qkv[h * 192 + 128:h * 192 + 192] for h in heads])
        wo = W_out[g * 256:(g + 1) * 256, :]                    # [256, 1024]

        xT = x[b].T.reshape(8, 128, S).transpose(1, 0, 2)       # [128, 8, S]
        wsh3 = wsh.reshape(8, 128, 768).transpose(1, 0, 2)      # [128, 8, 768]
        wo3 = wo.reshape(2, 128, D).transpose(1, 0, 2)          # [128, 2, D]
        bq2 = np.concatenate([bqv, bkv]).reshape(4, 128).T      # [128, 4]
        bv2 = np.broadcast_to(bvv, (128, 256))                  # [128, 256]

        in_maps.append({
            "xT": np.ascontiguousarray(xT).astype(ml_dtypes.bfloat16),
            "w": np.ascontiguousarray(wsh3).astype(ml_dtypes.bfloat16),
            "wout": np.ascontiguousarray(wo3).astype(ml_dtypes.bfloat16),
            "bq": np.ascontiguousarray(bq2),
            "bv": np.ascontiguousarray(bv2),
            "cmask": np.ascontiguousarray(cmask),
        })

    _CACHE["in_maps"] = in_maps
    res = bass_utils.run_bass_kernel_spmd(nc, in_maps, core_ids=list(range(N_CORES)))

    out = np.zeros((B, S, D), np.float32)
    for c in range(N_CORES):
        b = c // 4
        oT = np.asarray(res.results[c]["outT"]).astype(np.float32)  # [128, 8, S]
        out[b] += oT.transpose(1, 0, 2).reshape(D, S).T
    out += b_out
    return out


# revision 61
# speedup vs baseline: 1.3360x; 1.0036x over previous
"""Multi-head causal attention (B=2, S=2048, D=1024, H=16) on 8 TRN2 NeuronCores.

Sharding: core c in 0..7 handles batch b = c // 4 and local head group
g = c % 4 (global heads 4g .. 4g+3).  Tensor-parallel over heads: each core
computes its heads' Q/K/V projections, causal attention, and a partial
output projection (W_out rows for its heads).  Host sums the 4 partials per
batch and adds b_out.

v2: bf16 storage everywhere (f32 PSUM accumulation), 128-granularity causal
trim, k-sliced startup DMA with k-outer chunk-0 projection, and phase
interleaving: qkv(c+1) / outproj(c-1) matmul blocks are emitted as filler
between attention ki-steps of chunk c so the tensor engine never waits on
the activation-engine exp cadence.
"""

from contextlib import ExitStack

import numpy as np
import ml_dtypes

import concourse.bass as bass
import concourse.mybir as mybir
import concourse.tile as tile
from concourse import bass_utils

F32 = mybir.dt.float32
BF16 = mybir.dt.bfloat16
EXP = mybir.ActivationFunctionType.Exp

B, S, D, H = 2, 2048, 1024, 16
HD = D // H          # 64
HL = 4               # heads per core
N_CORES = 8
SC = S // 512        # 4 q-chunks of 512
KT = S // 128        # 16 k-tiles of 128

_CACHE = {}

_NO_HOIST = {
    "AllEngineBarrier",
    "EventSemaphore",
    "UnconditionalBranch",
    "CompareAndBranch",
    "BranchHint",
    "IndirectBranch",
    "Halt",
    "Call",
    "OverlayCall",
    "NoOp",
}


def _fix_sync_waits(nc):
    """walrus codegen holds only one sync-wait per engine instruction; hoist
    excess waits onto same-engine NoOps inserted right before."""
    for fn in nc.m.functions:
        for blk in fn.blocks:
            insts = blk.instructions
            out = []
            changed = False
            for inst in insts:
                si = inst.sync_info
                if si is not None and inst.opcode not in _NO_HOIST:
                    waits = list(si.on_wait)
                    if len(waits) > 1:
                        for j, w in enumerate(waits[:-1]):
                            nop = mybir.InstNoOp(name=f"{inst.name}-wfix{j}")
                            nop.engine = inst.engine
                            nop.sync_info = mybir.SyncInfo(on_wait=[w], on_update=[])
                            out.append(nop)
                        inst.sync_info = mybir.SyncInfo(
                            on_wait=[waits[-1]], on_update=list(si.on_update)
                        )
                        changed = True
                out.append(inst)
            if changed:
                blk.instructions = out


def _build(reps=1, fix_waits=True, n_chunks=SC, trim=True, masks=True):
    nc = bass.Bass("TRN2", target_bir_lowering=False, debug=False,
                   num_devices=N_CORES)

    xT = nc.dram_tensor("xT", [128, 8, S], BF16, kind="ExternalInput").ap()
    w = nc.dram_tensor("w", [128, 8, 768], BF16, kind="ExternalInput").ap()
    wout = nc.dram_tensor("wout", [128, 2, D], BF16, kind="ExternalInput").ap()
    bq = nc.dram_tensor("bq", [128, 4], F32, kind="ExternalInput").ap()
    bv = nc.dram_tensor("bv", [128, 256], F32, kind="ExternalInput").ap()
    cmask = nc.dram_tensor("cmask", [128, 128], BF16, kind="ExternalInput").ap()
    outT = nc.dram_tensor("outT", [128, 8, S], BF16, kind="ExternalOutput").ap()

    with tile.TileContext(nc) as tc, ExitStack() as ctx:
        persist = ctx.enter_context(tc.tile_pool(name="persist", bufs=1))
        xpool = ctx.enter_context(tc.tile_pool(name="xp", bufs=3))
        epool = ctx.enter_context(tc.tile_pool(name="ep", bufs=3))
        rpool = ctx.enter_context(tc.tile_pool(name="rp", bufs=2))
        opool = ctx.enter_context(tc.tile_pool(name="op", bufs=4))
        # PSUM: scores 2x[128,1024] (4 banks) + AV accum 2x[128,512] (2) +
        # shared qkv/outproj/rb pool 2x[128,512] (2) = 8 banks
        ps_sc = ctx.enter_context(tc.tile_pool(name="ps_sc", bufs=2, space="PSUM"))
        ps_av = ctx.enter_context(tc.tile_pool(name="ps_av", bufs=2, space="PSUM"))
        ps_mm = ctx.enter_context(tc.tile_pool(name="ps_mm", bufs=2, space="PSUM"))

        w_sb = persist.tile([128, 8, 768], BF16, tag="w")
        wout_sb = persist.tile([128, 2, D], BF16, tag="wout")
        bq_sb = persist.tile([128, 4], F32, tag="bq")
        bv_sb = persist.tile([128, 256], F32, tag="bv")
        sel_sb = persist.tile([128, 256], BF16, tag="sel")
        cmask_sb = persist.tile([128, 128], BF16, tag="cmask")
        qT = persist.tile([128, 2, S], BF16, tag="qT")
        kT = persist.tile([128, 2, S], BF16, tag="kT")
        vn = persist.tile([128, KT, HL, 97], BF16, tag="vn")
        vraw = persist.tile([128, 2, S], BF16, tag="vraw")
        vnorm = persist.tile([128, 2, S], BF16, tag="vnorm")

        # device-built constants: the softmax-denominator ones column of the
        # augmented V — col 64 for even heads (den -> po row 64), col 96 for
        # odd heads (den -> po row 96, a valid mod-32 partition base) — and
        # the K=33 reciprocal-broadcast selector (rows 65..95 all zero).
        for h in range(HL):
            if h % 2 == 0:
                nc.vector.memset(vn[:, :, h, 64:65], 1.0)
            else:
                nc.vector.memset(vn[:, :, h, 64:96], 0.0)
                nc.vector.memset(vn[:, :, h, 96:97], 1.0)
        nc.vector.memset(sel_sb[64:96, 0:128], 0.0)
        nc.vector.memset(sel_sb[64:65, 0:64], 1.0)
        nc.vector.memset(sel_sb[96:97, 0:64], 0.0)
        nc.vector.memset(sel_sb[96:97, 64:128], 1.0)
        recrs = [persist.tile([128, 512], BF16, tag=f"recr{i}", name=f"recr{i}")
                 for i in range(2)]
        for r in recrs:
            nc.vector.memset(r[64:96, :], 0.0)

        # ---- startup DMA, k-sliced so the first matmuls start early ----
        xc0 = xpool.tile([128, 8, 512], BF16, tag="xc", name="xc0")
        nc.sync.dma_start(xc0[:, 0, :], xT[:, 0, 0:512])
        nc.scalar.dma_start(w_sb[:, 0, 0:512], w[:, 0, 0:512])
        nc.sync.dma_start(xc0[:, 1, :], xT[:, 1, 0:512])
        nc.scalar.dma_start(w_sb[:, 0, 512:768], w[:, 0, 512:768])
        nc.scalar.dma_start(w_sb[:, 1, :], w[:, 1, :])
        for k2 in range(1, 4):
            ks2 = slice(2 * k2, 2 * k2 + 2)
            nc.sync.dma_start(xc0[:, ks2, :], xT[:, ks2, 0:512])
            nc.scalar.dma_start(w_sb[:, ks2, :], w[:, ks2, :])
        nc.scalar.dma_start(bq_sb[:], bq)
        nc.scalar.dma_start(bv_sb[:], bv)
        nc.scalar.dma_start(cmask_sb[:], cmask)
        nc.scalar.dma_start(wout_sb[:], wout)

        # ---- chunk-0 qkv projection, k-outer (consumes slices as they land)
        # spA: [Q hp0 | K hp0], spB: [Q hp1 | K hp1]; pvA: [j0|j1], pvB: [j2|j3]
        # (hardware: at most ONE open matmul accumulation group per PSUM bank
        # — interleaved groups in a shared bank silently corrupt, so the four
        # qk accumulators get a bank each and V runs as sequential j-blocks)
        spA = ps_sc.tile([128, 1024], F32, tag="s", name="spA")
        spB = ps_sc.tile([128, 1024], F32, tag="s", name="spB")
        for k in range(8):
            st = k == 0
            sp_ = k == 7
            nc.tensor.matmul(spA[:, 0:512], w_sb[:, k, 0:128], xc0[:, k, :],
                             start=st, stop=sp_)
            nc.tensor.matmul(spA[:, 512:1024], w_sb[:, k, 256:384], xc0[:, k, :],
                             start=st, stop=sp_)
            nc.tensor.matmul(spB[:, 0:512], w_sb[:, k, 128:256], xc0[:, k, :],
                             start=st, stop=sp_)
            nc.tensor.matmul(spB[:, 512:1024], w_sb[:, k, 384:512], xc0[:, k, :],
                             start=st, stop=sp_)
        nc.vector.tensor_scalar_add(qT[:, 0, 0:512], spA[:, 0:512], bq_sb[:, 0:1])
        nc.vector.tensor_scalar_add(kT[:, 0, 0:512], spA[:, 512:1024], bq_sb[:, 2:3])
        nc.vector.tensor_scalar_add(qT[:, 1, 0:512], spB[:, 0:512], bq_sb[:, 1:2])
        nc.vector.tensor_scalar_add(kT[:, 1, 0:512], spB[:, 512:1024], bq_sb[:, 3:4])
        for st4 in range(4):
            pv = ps_av.tile([128, 512], F32, tag="av", name=f"pv0_{st4}")
            for k in range(8):
                nc.tensor.matmul(pv[:, 0:256],
                                 xc0[:, k, 128 * st4:128 * (st4 + 1)],
                                 w_sb[:, k, 512:768], start=(k == 0), stop=(k == 7))
            nc.vector.tensor_add(
                vn[:, st4, :, 0:64],
                pv[:, 0:256].rearrange("p (h d) -> p h d", h=4),
                bv_sb[:].rearrange("p (h d) -> p h d", h=4))

        xcs = {0: xc0}

        # ---- filler blocks: qkv projection of a later chunk / output
        # projection of an earlier chunk, emitted between attention steps ----
        def qk_halves(cn, m):
            # m: 0=Q hp0, 1=Q hp1, 2=K hp0, 3=K hp1 (matches w col + bias col)
            # split into two 4-k-step closures for finer filler granularity
            hold = {}

            def emit_a():
                hold["pm"] = ps_mm.tile([128, 512], F32, tag="mm",
                                        name=f"qk{cn}_{m}")
                for k in range(4):
                    nc.tensor.matmul(hold["pm"][:], w_sb[:, k, 128 * m:128 * (m + 1)],
                                     xcs[cn][:, k, :], start=(k == 0), stop=False)

            def emit_b():
                qs = slice(cn * 512, (cn + 1) * 512)
                pm = hold["pm"]
                for k in range(4, 8):
                    nc.tensor.matmul(pm[:], w_sb[:, k, 128 * m:128 * (m + 1)],
                                     xcs[cn][:, k, :], start=False, stop=(k == 7))
                dst = qT[:, m, qs] if m < 2 else kT[:, m - 2, qs]
                nc.vector.tensor_scalar_add(dst, pm[:], bq_sb[:, m:m + 1])
            return [emit_a, emit_b]

        def v_block(cn, j):
            def emit():
                pv = ps_mm.tile([128, 512], F32, tag="mm", name=f"v{cn}_{j}")
                for k in range(8):
                    nc.tensor.matmul(pv[:, 0:256],
                                     xcs[cn][:, k, 128 * j:128 * (j + 1)],
                                     w_sb[:, k, 512:768], start=(k == 0), stop=(k == 7))
                st4 = 4 * cn + j
                nc.vector.tensor_add(
                    vn[:, st4, :, 0:64],
                    pv[:, 0:256].rearrange("p (h d) -> p h d", h=4),
                    bv_sb[:].rearrange("p (h d) -> p h d", h=4))
            return emit

        ou_hold = {}
        pu_hold = {}

        def outproj_block(cn, m, copy_eng="dve", psum="mm"):
            # even m allocates a 2-block staging tile; odd m completes it and
            # issues one paired DMA (halves the per-transfer HWDGE overhead).
            # psum="sc": after the last scores, the 4 score banks are free —
            # pair two blocks per [128,1024] tile for deeper PU buffering.
            def emit():
                qs = slice(cn * 512, (cn + 1) * 512)
                if psum == "mm":
                    pu = ps_mm.tile([128, 512], F32, tag="mm",
                                    name=f"pu{cn}_{m}")[:]
                else:
                    if m % 2 == 0:
                        pu_hold[cn] = ps_sc.tile([128, 1024], F32, tag="s",
                                                 name=f"pu2{cn}_{m}")
                    pu = pu_hold[cn][:, 512 * (m % 2):512 * (m % 2) + 512]
                for t in range(2):
                    nc.tensor.matmul(pu, wout_sb[:, t, 128 * m:128 * (m + 1)],
                                     vnorm[:, t, qs], start=(t == 0), stop=(t == 1))
                if m % 2 == 0:
                    ou_hold[cn] = opool.tile([128, 2, 512], BF16, tag="ou",
                                             name=f"ou{cn}_{m}")
                ou = ou_hold[cn]
                dst = ou[:, m % 2, :]
                if copy_eng == "dve":
                    nc.vector.tensor_copy(dst, pu)
                elif copy_eng == "act":
                    nc.scalar.copy(dst, pu)
                if m % 2 == 1:
                    nc.sync.dma_start(outT[:, m - 1:m + 1, qs], ou[:])
            return emit

        # pending normalization chain of the previous head pair — emitted
        # right AFTER the next head pair's first exp is queued, so the Act
        # engine starts the next exp before the drain copies, and the PE has
        # scores/filler work while the reciprocal chain flows (crosses chunk
        # boundaries too)
        pending = {"norm": None}

        # ---- attention for one q-chunk, with filler drained between steps
        def attn_qc(qc, fillers, reserve=()):
            n_ki = 4 * qc + 4
            nsteps = 2 * n_ki + 2
            state = {"step": 0, "drained": 0}

            def tick(n=None):
                state["step"] += 1
                if n is None:
                    # at least one filler by step 1: right after a head-pair
                    # boundary the PE otherwise idles on the exp/recip chains
                    target = max(len(fillers) * state["step"] // nsteps,
                                 min(1, state["step"]))
                else:
                    target = state["drained"] + n
                while state["drained"] < min(target, len(fillers)):
                    fillers[state["drained"]]()
                    state["drained"] += 1

            qs = slice(qc * 512, (qc + 1) * 512)
            for hp in range(2):
                # po tiles are allocated lazily at the first AV so the pool
                # WAR lands after the previous pair's (deferred) drain copies
                po = [None, None]
                recr = recrs[(2 * qc + hp) % 2]
                es_hold = [None] * n_ki

                def do_av(ki, qc=qc, hp=hp, po=po, n_ki=n_ki, es_hold=es_hold):
                    if po[0] is None:
                        po[0] = ps_av.tile([128, 512], F32, tag="av",
                                           name=f"po{qc}{hp}0")
                        po[1] = ps_av.tile([128, 512], F32, tag="av",
                                           name=f"po{qc}{hp}1")
                    e, o, wdt = es_hold[ki]
                    for i in range(2):
                        h = 2 * hp + i
                        # V_aug.T @ E: rows 0..63 values, row 64 (even) or 96
                        # (odd) the softmax denominator (ones column of V_aug)
                        nc.tensor.matmul(
                            po[i][0:65 + 32 * i, o:512], vn[:, ki, h, 0:65 + 32 * i],
                            e[:, i * 512:i * 512 + wdt],
                            start=(ki == 0), stop=(ki == n_ki - 1),
                            skip_group_check=True)

                for ki in range(n_ki):
                    j = ki - 4 * qc
                    o = 128 * j if (trim and j >= 0) else 0
                    wdt = 512 - o
                    ks = slice(ki * 128, (ki + 1) * 128)
                    qsub = slice(qc * 512 + o, (qc + 1) * 512)
                    # head slabs live at bank-aligned offsets i*512 — the two
                    # tile_position score groups must not share a PSUM bank
                    sp = ps_sc.tile([128, 1024], F32, tag="s",
                                    name=f"sp{qc}{hp}{ki}")
                    for i in range(2):   # head within pair (row-packed)
                        vp = 64 * i
                        nc.tensor.matmul(
                            sp[:, i * 512:i * 512 + wdt],
                            kT[vp:vp + 64, hp, ks], qT[vp:vp + 64, hp, qsub],
                            start=True, stop=True, tile_position=(vp, 0))
                    e = epool.tile([128, 1024], BF16, tag="e",
                                   name=f"e{qc}{hp}{ki}")
                    if wdt == 512:
                        nc.scalar.activation(e[:], sp[:], EXP, scale=0.125)
                    else:
                        sp3 = sp[:].rearrange("p (t q) -> p t q", t=2)
                        e3 = e[:].rearrange("p (t q) -> p t q", t=2)
                        nc.scalar.activation(e3[:, :, 0:wdt], sp3[:, :, 0:wdt],
                                             EXP, scale=0.125)
                    if masks and j >= 0:
                        # diagonal tile: with o=128j the invalid region is
                        # always the leading 128-col triangle (kk > qq)
                        mw = min(128, wdt)
                        for i in range(2):
                            es = e[:, i * 512:i * 512 + mw]
                            nc.vector.tensor_mul(es, es, cmask_sb[:, 0:mw])
                    es_hold[ki] = (e, o, wdt)
                    if ki == 0 and pending["norm"] is not None:
                        pending["norm"]()
                        pending["norm"] = None
                    if ki >= 1:
                        do_av(ki - 1)
                    tick()
                do_av(n_ki - 1)
                tick()

                # normalization: per-head reciprocal of the denominator rows,
                # broadcast onto value partitions via one K=33 selector matmul.
                # Denominators: even head on po[0] row 64, odd head on po[1]
                # row 96 (the odd V_aug ones column sits at col 96), so both
                # reciprocals stay partition-aligned (rows 65..95 are zero).
                def norm(qc=qc, hp=hp, po=po, recr=recr, qs=qs,
                         res=tuple(reserve) if (hp == 1 and reserve) else ()):
                    stage = rpool.tile([128, 512], BF16, tag="stage",
                                       name=f"st{qc}{hp}")
                    nc.scalar.copy(stage[0:64, :], po[1][0:64, :])
                    # sync ring: out-DMAs queued behind this were emitted in
                    # the same window, so the short stage-copy wait cannot
                    # head-of-line-block them for long
                    nc.sync.dma_start(vraw[64:128, hp, qs], stage[0:64, :])
                    with nc.allow_low_precision(reason="bf16 softmax recip"):
                        nc.vector.reciprocal(recr[96:97, :], po[1][96:97, :])
                        nc.vector.reciprocal(recr[64:65, :], po[0][64:65, :])
                    nc.scalar.copy(vraw[0:64, hp, qs], po[0][0:64, :])
                    if res:
                        res[0]()
                        res[1]()
                    else:
                        tick(n=1)
                    rb = ps_mm.tile([128, 512], F32, tag="mm", name=f"rb{qc}{hp}")
                    nc.tensor.matmul(rb[:], sel_sb[64:97, 0:128], recr[64:97, :],
                                     start=True, stop=True, tile_position=(64, 0))
                    for r in res[2:]:
                        r()
                    nc.vector.tensor_mul(vnorm[:, hp, qs], vraw[:, hp, qs], rb[:])
                pending["norm"] = norm
            # any fillers not yet drained
            while state["drained"] < len(fillers):
                fillers[state["drained"]]()
                state["drained"] += 1

        # ---- main schedule: attn(c) with qkv(c+1) as filler; all deferrable
        # output projections (chunks 0..2) land in attn(3), whose exp cadence
        # otherwise starves the tensor engine; outproj(3) is the tail.  In
        # attn(3) the copies ride the idle Pool engine so the DVE recip ->
        # vnorm chain stays short; the last few blocks are reserved to keep
        # the PE warm through the final normalization chain.
        for c in range(n_chunks):
            fillers, reserve = [], []
            if c + 1 < n_chunks:
                xc = xpool.tile([128, 8, 512], BF16, tag="xc", name=f"xc{c+1}")
                qsn = slice((c + 1) * 512, (c + 2) * 512)
                nc.sync.dma_start(xc[:, 0:4, :], xT[:, 0:4, qsn])
                nc.sync.dma_start(xc[:, 4:8, :], xT[:, 4:8, qsn])
                xcs[c + 1] = xc
                for m in range(4):
                    fillers += qk_halves(c + 1, m)
                fillers += [v_block(c + 1, j) for j in range(4)]
            else:
                # GPSIMD cannot touch PSUM, so drain copies ride DVE (fillers)
                # and Act (reserve, where the exp pipeline has already drained)
                for cn in range(n_chunks - 2):
                    fillers += [outproj_block(cn, m, "dve") for m in range(8)]
                fillers += [outproj_block(n_chunks - 2, m, "dve") for m in range(4)]
                reserve = [outproj_block(n_chunks - 2, m, "act") for m in range(4, 8)]
            attn_qc(c, fillers, reserve)
        # the last head pair's normalization, then the tail output projection
        pending["norm"]()
        pending["norm"] = None
        for m in range(8):
            outproj_block(n_chunks - 1, m, copy_eng=("dve" if m % 2 else "act"))()

    if fix_waits:
        _fix_sync_waits(nc)
    return nc


def _get_nc():
    if "nc" not in _CACHE:
        _CACHE["nc"] = _build()
    return _CACHE["nc"]


def _make_cmask() -> np.ndarray:
    """cmask[128, 128]: c[kk, qq] = 1 iff kk <= qq (relative causal triangle
    applied to the leading 128 cols of every diagonal score tile)."""
    kk = np.arange(128)[:, None]
    qq = np.arange(128)[None, :]
    return (kk <= qq).astype(np.float32)


def kernel(x, W_qkv, b_qkv, W_out, b_out):
    x = np.asarray(x, np.float32)
    W_qkv = np.asarray(W_qkv, np.float32)
    b_qkv = np.asarray(b_qkv, np.float32)
    W_out = np.asarray(W_out, np.float32)
    b_out = np.asarray(b_out, np.float32)

    nc = _get_nc()
    cmask = _make_cmask().astype(ml_dtypes.bfloat16)

    in_maps = []
    for c in range(N_CORES):
        b, g = divmod(c, 4)
        heads = [4 * g + i for i in range(HL)]
        # reorder W_qkv columns: [Q(h0..h3) | K(h0..h3) | V(h0..h3)]
        qcols = np.concatenate([W_qkv[:, h * 192:h * 192 + 64] for h in heads], 1)
        kcols = np.concatenate([W_qkv[:, h * 192 + 64:h * 192 + 128] for h in heads], 1)
        vcols = np.concatenate([W_qkv[:, h * 192 + 128:h * 192 + 192] for h in heads], 1)
        wsh = np.concatenate([qcols, kcols, vcols], 1)          # [1024, 768]
        bqv = np.concatenate([b_qkv[h * 192:h * 192 + 64] for h in heads])
        bkv = np.concatenate([b_qkv[h * 192 + 64:h * 192 + 128] for h in heads])
        bvv = np.concatenate([b_qkv[h * 192 + 128:h * 192 + 192] for h in heads])
        wo = W_out[g * 256:(g + 1) * 256, :]                    # [256, 1024]

        xT = x[b].T.reshape(8, 128, S).transpose(1, 0, 2)       # [128, 8, S]
        wsh3 = wsh.reshape(8, 128, 768).transpose(1, 0, 2)      # [128, 8, 768]
        wo3 = wo.reshape(2, 128, D).transpose(1, 0, 2)          # [128, 2, D]
        bq2 = np.concatenate([bqv, bkv]).reshape(4, 128).T      # [128, 4]
        bv2 = np.broadcast_to(bvv, (128, 256))                  # [128, 256]

        in_maps.append({
            "xT": np.ascontiguousarray(xT).astype(ml_dtypes.bfloat16),
            "w": np.ascontiguousarray(wsh3).astype(ml_dtypes.bfloat16),
            "wout": np.ascontiguousarray(wo3).astype(ml_dtypes.bfloat16),
            "bq": np.ascontiguousarray(bq2),
            "bv": np.ascontiguousarray(bv2),
            "cmask": np.ascontiguousarray(cmask),
        })

    _CACHE["in_maps"] = in_maps
    res = bass_utils.run_bass_kernel_spmd(nc, in_maps, core_ids=list(range(N_CORES)))

    out = np.zeros((B, S, D), np.float32)
    for c in range(N_CORES):
        b = c // 4
        oT = np.asarray(res.results[c]["outT"]).astype(np.float32)  # [128, 8, S]
        out[b] += oT.transpose(1, 0, 2).reshape(D, S).T
    out += b_out
    return out
